# revision 7
# baseline (speedup 1.0000x reference)
"""Trainium2 Bass kernel for nn_GameboyNet (sparse windowed attention net).

Sharding: pure data-parallel over batch — B=8 rows, one per NeuronCore.
Each core runs the full 32-layer network on its own (S=4096, D=256)
sequence, residual stream resident in SBUF feature-major (2x128 x 4096)
f32.

All heavy matmuls run in fp8e4 with MatmulPerfMode.DoubleRow (K=256 per
instruction, 2x PE throughput). Numerics hold because per-element fp8
noise is random and averages out over the K=256..1024 contractions; the
residual stream stays f32.

Algebraic folds (host-side, exact):
 - BatchNorm (eval, affine) never materializes in-kernel: a per-feature
   affine (P, Q) is tracked across layers and folded into every weight
   and bias. Additive constants (v-bias, W2 colsum, b2) ride through
   softmax (weights sum to 1) and residual adds, so they fold into Q too.
 - k-bias vanishes: softmax is invariant to per-query/constant score
   offsets, so only q needs its bias.
 - Softmax denominator comes from an all-ones fp8 stationary (value 2^5),
   giving a partition-broadcast row-sum for free; reciprocal via the fast
   custom-DVE approx (~18 bits) directly yields acc * (1/ssum) scaling.
"""
import os
import sys
import types

sys.path.insert(0, '/opt/trn_rl_repo')

import numpy as np
import ml_dtypes

import concourse.bass as bass
import concourse.mybir as mybir
import concourse.tile as tile
from concourse import bacc
from concourse.bass import ds
from concourse.bass_utils import run_bass_kernel_spmd

B, S, D, W, L = 8, 4096, 256, 512, 32
E = 4 * D
NW = S // W
P = 128
DC = D // P          # 2 d-chunks
EC = E // P          # 8 e-chunks
TT = S // 512        # 8 token tiles of 512
TB = S // P          # 32 token blocks of 128
BN_EPS = 1e-5
NEG = -1e9

f32 = mybir.dt.float32
bf16 = mybir.dt.bfloat16
f8 = mybir.dt.float8e4
AF = mybir.ActivationFunctionType
ALU = mybir.AluOpType
DR = mybir.MatmulPerfMode.DoubleRow

# power-of-two quantization scales (dequants folded into op scalars)
WS = 2.0 ** 9     # weights
HS = 2.0 ** 5     # residual h -> fp8
AQ = 2.0 ** 3     # q store scale (1/sqrt(D)=1/16 folded into exp dequant)
AK = 2.0 ** 4     # k store scale
AV_ = 2.0 ** 5    # v store scale (ssum ones match it so recip folds)
SEXP = 2.0 ** -11         # exp dequant: -(3+4+4)
SQ = 2.0 ** -11           # pq -> q*: 2^(3-14)
SK = 2.0 ** -10           # pk -> k*: 2^(4-14)
SV = 2.0 ** -9            # pv -> v*: 2^(5-14)
SU = 2.0 ** -15           # pu -> tanh arg: 0.5 * 2^-14
SW2 = 2.0 ** -9           # pm -> resid: 1/WS

LAST_EXEC_NS = None
LAST_TRACE = None

_cache = {}


def _install_ntff_hook():
    """The agent image's antenv is a stub without axon_hooks; inject it so
    trace=True can capture NTFF profiles through the axon tunnel."""
    try:
        import antenv
        if 'antenv.axon_hooks' in sys.modules:
            return
        mod = types.ModuleType("antenv.axon_hooks")
        _HOOK = [None]
        mod.set_axon_ntff_profile_hook = lambda h: _HOOK.__setitem__(0, h)
        mod.get_axon_ntff_profile_hook = lambda: _HOOK[0]
        sys.modules["antenv.axon_hooks"] = mod
        antenv.axon_hooks = mod
        from trn_agent_boot.trn_boot import _ntff_profile_via_ctypes
        hook = _ntff_profile_via_ctypes('/opt/axon/libaxon_pjrt.so')
        mod.set_axon_ntff_profile_hook(hook)
    except Exception:
        pass


def _emit_layer(nc, tc, pools, l):
    (wpool, psum, usb, tmpp, rbp,
     hT, hf8, qT, kT, vtm, expbufs, ones_dr, maskT) = pools

    dma = nc.sync.dma_start

    # ---- per-layer weight loads (single DMA each, fp8) ----------------
    wq_sb = wpool.tile([P, DC, D], f8, tag="wq")
    wk_sb = wpool.tile([P, DC, D], f8, tag="wk")
    wv_sb = wpool.tile([P, DC, D], f8, tag="wv")
    w1_sb = wpool.tile([P, DC, E], f8, tag="w1")
    w2_sb = wpool.tile([P, EC, D], f8, tag="w2")
    cons = wpool.tile([P, 10], f32, tag="cons")
    rows = ds(l * P, P)
    dma(out=wq_sb, in_=nc.t_wq[rows, :])
    dma(out=wk_sb, in_=nc.t_wk[rows, :])
    dma(out=wv_sb, in_=nc.t_wv[rows, :])
    dma(out=w1_sb, in_=nc.t_w1[rows, :])
    dma(out=w2_sb, in_=nc.t_w2[rows, :])
    dma(out=cons, in_=nc.t_cons[rows, :])
    # cons: 0:2 = 8*bq' per oc, 2:10 = 0.5*b1' per ec

    # ---- cast h -> fp8 (phase A, gpsimd: SBUF->SBUF) ------------------
    for tt in range(TT):
        tsl = slice(tt * 512, (tt + 1) * 512)
        nc.gpsimd.tensor_scalar(hf8[:, :, tsl], hT[:, :, tsl], HS, None,
                                op0=ALU.mult)

    # ---- QKV (DoubleRow fp8) ------------------------------------------
    for oc in range(DC):
        ocs = slice(oc * P, (oc + 1) * P)
        for tt in range(TT):
            tsl = slice(tt * 512, (tt + 1) * 512)
            pq = psum.tile([P, 512], f32, tag="ps")
            nc.tensor.matmul(pq[:], wq_sb[:, :, ocs], hf8[:, :, tsl],
                             start=True, stop=True, perf_mode=DR)
            nc.vector.tensor_scalar(qT[:, oc, tsl], pq[:], SQ,
                                    cons[:, oc:oc + 1], op0=ALU.mult,
                                    op1=ALU.add)
            pk = psum.tile([P, 512], f32, tag="ps")
            nc.tensor.matmul(pk[:], wk_sb[:, :, ocs], hf8[:, :, tsl],
                             start=True, stop=True, perf_mode=DR)
            nc.vector.tensor_scalar(kT[:, oc, tsl], pk[:], SK, None,
                                    op0=ALU.mult)
    # v token-major [t, d]
    for tb in range(TB):
        pv = psum.tile([P, 512], f32, tag="ps")
        nc.tensor.matmul(pv[:, 0:D], hf8[:, :, tb * P:(tb + 1) * P],
                         wv_sb[:, :, :], start=True, stop=True, perf_mode=DR)
        nc.vector.tensor_scalar(vtm[:, tb, :], pv[:, 0:D], SV, None,
                                op0=ALU.mult)

    # ---- attention -----------------------------------------------------
    for w in range(NW):
        q0 = w * W
        expT = expbufs[w % 2]
        kb_lo = 4 if w == 0 else 0
        kstart = (w - 1) * W
        for kb in range(kb_lo, 8):
            kpos = kstart + kb * P
            qlo = 0 if kb < 4 else (kb - 4) * P
            ps = psum.tile([P, 512], f32, tag="ps")
            nc.tensor.matmul(ps[:, qlo:512], kT[:, :, kpos:kpos + P],
                             qT[:, :, q0 + qlo:q0 + 512],
                             start=True, stop=True, perf_mode=DR)
            if kb >= 4:
                nc.vector.tensor_tensor(ps[:, qlo:qlo + P], ps[:, qlo:qlo + P],
                                        maskT[:, :], op=ALU.add)
            nc.scalar.activation(expT[:, kb, qlo:512], ps[:, qlo:512],
                                 AF.Exp, scale=SEXP)
        acc0 = psum.tile([P, 512], f32, tag="ps")
        acc1 = psum.tile([P, 512], f32, tag="ps")
        accs = [acc0, acc1]
        ssb = psum.tile([P, 512], f32, tag="ps")
        for jj in range(4):
            tb0 = max((w - 1) * 4 + 2 * jj, 0)
            first, last = (jj == 0), (jj == 3)
            esl = expT[:, 2 * jj:2 * jj + 2, :]
            for dc in range(DC):
                nc.tensor.matmul(accs[dc][:], vtm[:, tb0:tb0 + 2, dc * P:(dc + 1) * P],
                                 esl, start=first, stop=last, perf_mode=DR,
                                 skip_group_check=True)
            nc.tensor.matmul(ssb[:], ones_dr[:, :, :], esl,
                             start=first, stop=last, perf_mode=DR,
                             skip_group_check=True)
        recip = rbp.tile([P, 512], f32, tag="recip")
        nc.vector.reciprocal_approx_fast(out=recip[:], in_=ssb[:])
        for dc in range(DC):
            tmp = tmpp.tile([P, 512], f32, tag="tmp")
            nc.vector.tensor_tensor(tmp[:], accs[dc][:], recip[:], op=ALU.mult)
            nc.gpsimd.tensor_tensor(hT[:, dc, q0:q0 + W], hT[:, dc, q0:q0 + W],
                                    tmp[:], op=ALU.add)

    # ---- MLP (DoubleRow fp8, tanh-centered) ---------------------------
    for tt in range(TT):
        tsl = slice(tt * 512, (tt + 1) * 512)
        nc.gpsimd.tensor_scalar(hf8[:, :, tsl], hT[:, :, tsl], HS, None,
                                op0=ALU.mult)
    for tt in range(TT):
        tsl = slice(tt * 512, (tt + 1) * 512)
        u_sb = usb.tile([P, EC, 512], f8, tag="u")
        for ec in range(EC):
            pu = psum.tile([P, 512], f32, tag="ps")
            nc.tensor.matmul(pu[:], w1_sb[:, :, ec * P:(ec + 1) * P],
                             hf8[:, :, tsl], start=True, stop=True,
                             perf_mode=DR)
            nc.scalar.activation(u_sb[:, ec, :], pu[:], AF.Tanh,
                                 bias=cons[:, 2 + ec:3 + ec], scale=SU)
        for dc in range(DC):
            pm = psum.tile([P, 512], f32, tag="ps")
            for jj in range(4):
                nc.tensor.matmul(pm[:], w2_sb[:, 2 * jj:2 * jj + 2, dc * P:(dc + 1) * P],
                                 u_sb[:, 2 * jj:2 * jj + 2, :],
                                 start=(jj == 0), stop=(jj == 3), perf_mode=DR)
            nc.vector.scalar_tensor_tensor(hT[:, dc, tsl], pm[:], SW2,
                                           hT[:, dc, tsl],
                                           op0=ALU.mult, op1=ALU.add)


def _build(n_layers=L):
    nc = bacc.Bacc("TRN2", target_bir_lowering=False, debug=False)

    h0_d = nc.dram_tensor("h0T", [D, S], f32, kind="ExternalInput")
    nc.t_wq = nc.dram_tensor("wq8", [n_layers * P, DC * D], f8, kind="ExternalInput")
    nc.t_wk = nc.dram_tensor("wk8", [n_layers * P, DC * D], f8, kind="ExternalInput")
    nc.t_wv = nc.dram_tensor("wv8", [n_layers * P, DC * D], f8, kind="ExternalInput")
    nc.t_w1 = nc.dram_tensor("w18", [n_layers * P, DC * E], f8, kind="ExternalInput")
    nc.t_w2 = nc.dram_tensor("w28", [n_layers * P, EC * D], f8, kind="ExternalInput")
    nc.t_cons = nc.dram_tensor("cons", [n_layers * P, 10], f32, kind="ExternalInput")
    mask_d = nc.dram_tensor("maskT", [P, P], f32, kind="ExternalInput")
    wfT_d = nc.dram_tensor("wfT", [D, D], bf16, kind="ExternalInput")
    bf_d = nc.dram_tensor("bfc", [P, DC], f32, kind="ExternalInput")
    out_d = nc.dram_tensor("outT", [D, S], f32, kind="ExternalOutput")

    with tile.TileContext(nc) as tc:
        with tc.tile_pool(name="persist", bufs=1) as persist, \
             tc.tile_pool(name="wpool", bufs=2) as wpool, \
             tc.tile_pool(name="psum", bufs=8, space="PSUM") as psum, \
             tc.tile_pool(name="usb", bufs=2) as usb, \
             tc.tile_pool(name="tmpp", bufs=3) as tmpp, \
             tc.tile_pool(name="rbp", bufs=2) as rbp, \
             tc.tile_pool(name="outp", bufs=4) as outp:

            hT = persist.tile([P, DC, S], f32)
            hf8 = persist.tile([P, DC, S], f8)
            qT = persist.tile([P, DC, S], f8)
            kT = persist.tile([P, DC, S], f8)
            vtm = persist.tile([P, TB, D], f8)
            expA = persist.tile([P, 8, 512], f8)
            expB = persist.tile([P, 8, 512], f8)
            ones_dr = persist.tile([P, DC, P], f8)
            maskT = persist.tile([P, P], f32)
            wf_sb = persist.tile([P, DC, D], bf16)
            bf_sb = persist.tile([P, DC], f32)
            hbf = persist.tile([P, DC, S], bf16)

            nc.vector.memset(ones_dr, AV_)
            nc.vector.memset(expA, 0.0)
            nc.vector.memset(expB, 0.0)
            nc.sync.dma_start(out=maskT, in_=mask_d[:, :])
            for kc in range(DC):
                nc.sync.dma_start(out=hT[:, kc, :], in_=h0_d[kc * P:(kc + 1) * P, :])
                nc.sync.dma_start(out=wf_sb[:, kc, :], in_=wfT_d[kc * P:(kc + 1) * P, :])
            nc.sync.dma_start(out=bf_sb, in_=bf_d[:, :])

            pools = (wpool, psum, usb, tmpp, rbp,
                     hT, hf8, qT, kT, vtm, (expA, expB), ones_dr, maskT)

            for l in range(n_layers):
                _emit_layer(nc, tc, pools, l)

            # final 1x1 conv + relu (bf16 for accuracy), feature-major out
            for tt in range(TT):
                tsl = slice(tt * 512, (tt + 1) * 512)
                nc.vector.tensor_copy(out=hbf[:, :, tsl], in_=hT[:, :, tsl])
            for oc in range(DC):
                for tt in range(TT):
                    tsl = slice(tt * 512, (tt + 1) * 512)
                    pf = psum.tile([P, 512], f32, tag="ps")
                    for kc in range(DC):
                        nc.tensor.matmul(pf[:], wf_sb[:, kc, oc * P:(oc + 1) * P],
                                         hbf[:, kc, tsl],
                                         start=(kc == 0), stop=(kc == DC - 1))
                    ot = outp.tile([P, 512], f32, tag="out")
                    nc.scalar.activation(ot[:], pf[:], AF.Relu,
                                         bias=bf_sb[:, oc:oc + 1])
                    nc.sync.dma_start(out=out_d[oc * P:(oc + 1) * P, tsl], in_=ot[:])

    nc.compile()
    return nc


def _dr_weight(Wmat, scale):
    """[out_dim, in_dim] f64 -> fp8 [128, in_dim//128, out_dim] flattened
    to [128, (in//128)*out]: [p, c, m] = W[m, c*128+p] * scale."""
    out_dim, in_dim = Wmat.shape
    c = in_dim // P
    w = (Wmat.T * scale).astype(np.float32)          # [in, out]
    w = w.reshape(c, P, out_dim).transpose(1, 0, 2)  # [p, c, m]
    return np.ascontiguousarray(w.reshape(P, c * out_dim)).astype(
        ml_dtypes.float8_e4m3)


def _prep_host(inputs, n_layers=L):
    bfl = ml_dtypes.bfloat16
    x = np.asarray(inputs['x'])
    emb = np.asarray(inputs['emb'], np.float32)
    bn_scale = 1.0 / np.sqrt(1.0 + BN_EPS)

    Wq = np.asarray(inputs['Wq'], np.float64)[:n_layers]
    Wk = np.asarray(inputs['Wk'], np.float64)[:n_layers]
    Wv = np.asarray(inputs['Wv'], np.float64)[:n_layers]
    W1 = np.asarray(inputs['W1'], np.float64)[:n_layers]
    W2 = np.asarray(inputs['W2'], np.float64)[:n_layers]
    bq = np.asarray(inputs['bq'], np.float64)[:n_layers]
    bv = np.asarray(inputs['bv'], np.float64)[:n_layers]
    b1 = np.asarray(inputs['b1'], np.float64)[:n_layers]
    b2 = np.asarray(inputs['b2'], np.float64)[:n_layers]
    gamma = np.asarray(inputs['gamma'], np.float64)[:n_layers]
    beta = np.asarray(inputs['beta'], np.float64)[:n_layers]

    wq8 = np.empty((n_layers * P, DC * D), ml_dtypes.float8_e4m3)
    wk8 = np.empty_like(wq8)
    wv8 = np.empty_like(wq8)
    w18 = np.empty((n_layers * P, DC * E), ml_dtypes.float8_e4m3)
    w28 = np.empty((n_layers * P, EC * D), ml_dtypes.float8_e4m3)
    cons = np.zeros((n_layers, P, 10), np.float32)

    Pv = np.ones(D)
    Q = np.zeros(D)
    for l in range(n_layers):
        A = gamma[l] * bn_scale
        C = beta[l]
        rows = slice(l * P, (l + 1) * P)
        wq8[rows] = _dr_weight(Wq[l] * Pv[None, :], WS)
        wk8[rows] = _dr_weight(Wk[l] * Pv[None, :], WS)
        wv8[rows] = _dr_weight(Wv[l] * Pv[None, :] / Pv[:, None], WS)
        w18[rows] = _dr_weight(W1[l] * Pv[None, :], WS)
        w28[rows] = _dr_weight(0.5 * W2[l] / Pv[:, None], WS)
        bq_f = (Wq[l] @ Q + bq[l])            # q bias (true units)
        cv0 = Wv[l] @ Q + bv[l]               # v-bias const, rides softmax
        Qmid = Q + cv0                        # offset after attention resid
        b1_f = (W1[l] @ Qmid + b1[l])
        cons[l, :, 0:2] = (bq_f * AQ).reshape(DC, P).T
        cons[l, :, 2:10] = (0.5 * b1_f).reshape(EC, P).T
        # affine absorb: v-bias consts + mlp consts + BN
        Q = A * (Qmid + 0.5 * W2[l].sum(axis=1) + b2[l]) + C
        Pv = A * Pv

    r = np.arange(P)
    maskT = np.where(r[None, :] >= r[:, None], 0.0, NEG).astype(np.float32)

    Wf = np.asarray(inputs['Wf'], np.float64)
    bfv = np.asarray(inputs['bf'], np.float64)
    wfT = np.ascontiguousarray((Wf * Pv[None, :]).T.astype(np.float32)).astype(bfl)
    bf_f = (Wf @ Q + bfv).astype(np.float32)
    bfc = bf_f.reshape(DC, P).T.copy()        # (P, DC)

    shared = dict(wq8=wq8, wk8=wk8, wv8=wv8, w18=w18, w28=w28,
                  cons=cons.reshape(n_layers * P, 10),
                  maskT=maskT, wfT=wfT, bfc=bfc)

    h0 = emb[x]                                # (B, S, D) f32
    in_maps = []
    for b in range(B):
        m = dict(shared)
        m['h0T'] = np.ascontiguousarray(h0[b].T)   # (D, S) f32
        in_maps.append(m)
    return in_maps


def kernel(**inputs):
    global LAST_EXEC_NS, LAST_TRACE
    n_layers = int(os.environ.get('KERNEL_NLAYERS', L))
    trace = os.environ.get('KERNEL_TRACE', '0') == '1'
    if trace:
        _install_ntff_hook()

    key = n_layers
    if key not in _cache:
        _cache[key] = _build(n_layers=n_layers)
    nc = _cache[key]

    in_maps = _prep_host(inputs, n_layers=n_layers)
    res = run_bass_kernel_spmd(nc, in_maps, core_ids=list(range(B)), trace=trace)
    LAST_EXEC_NS = res.exec_time_ns
    LAST_TRACE = res.instructions_and_trace[1] if res.instructions_and_trace else None
    out = np.stack([res.results[b]['outT'] for b in range(B)], axis=0)
    return out


# revision 13
# speedup vs baseline: 2.5461x; 2.5461x over previous
"""Trainium2 Bass kernel for nn_GameboyNet (sparse windowed attention net).

Sharding: pure data-parallel over batch — B=8 rows, one per NeuronCore.
Residual stream hT stays f32 in SBUF, feature-major [128, 2, 4096].
All heavy matmuls run fp8e4 DoubleRow (K=256/instruction, 2x PE rate).

Structural folds (host-side, exact):
 - BatchNorm + all additive constants fold into a tracked per-feature
   affine (P, Q); biases that ride softmax/residual vanish into Q.
 - Q/K projections merge: scores = h_k^T G h_q with G = Wk^T Wq / 16
   precomputed on host; the k-side bias c = Wk^T bq / 16 is applied in
   the u-drain; q-side bias terms cancel in softmax. One projection +
   one PSUM drain instead of two, and the scores moving operand is the
   already-resident hf8.
 - h->fp8 casts and the V-projection PSUM drain go through gpsimd
   software-DGE DMAs (the only cast-capable DMA path), freeing DVE.
   Wv carries its own 2^5 scale so the drain needs no multiply.
 - Softmax denominator: all-ones fp8 stationary (value 2^5) broadcasts
   row sums to all partitions; reciprocal_approx_fast folds the 2^-5.

PSUM is carved into four 2-bank [128,1024] slots (single tag) so exp and
tanh batch two banks per ACT instruction and the W2 residual/norm use
wide [128,2,512] DVE ops.
"""
import os
import sys
import types

sys.path.insert(0, '/opt/trn_rl_repo')

import numpy as np
import ml_dtypes

import concourse.bass as bass
import concourse.mybir as mybir
import concourse.tile as tile
from concourse import bacc
from concourse.bass import ds
from concourse.bass_utils import run_bass_kernel_spmd

B, S, D, W, L = 8, 4096, 256, 512, 32
E = 4 * D
NW = S // W
P = 128
DC = D // P          # 2 d-chunks
EC = E // P          # 8 e-chunks
TT = S // 512        # 8 token tiles of 512
TB = S // P          # 32 token blocks of 128
BN_EPS = 1e-5
NEG = -1e9

f32 = mybir.dt.float32
bf16 = mybir.dt.bfloat16
f8 = mybir.dt.float8e4
AF = mybir.ActivationFunctionType
ALU = mybir.AluOpType
DR = mybir.MatmulPerfMode.DoubleRow

# scales (h is fp8 at scale 1)
WG = 2.0 ** 13    # G (merged qk) weight scale
AU = 2.0 ** 10    # u store scale
WVS = 2.0 ** 5    # Wv scale == v store scale (drain is a pure DMA cast)
WS = 2.0 ** 9     # W1 / W2 scale
SU_DR = AU / WG           # u drain: pu * 2^-6 (+ c bias)
SEXP = 1.0 / AU           # exp dequant 2^-10
STANH = 0.5 / WS          # tanh scale 2^-10
SW2 = 1.0 / WS            # W2 resid dequant 2^-9

LAST_EXEC_NS = None
LAST_TRACE = None

_cache = {}


def _install_ntff_hook():
    try:
        import antenv
        if 'antenv.axon_hooks' in sys.modules:
            return
        mod = types.ModuleType("antenv.axon_hooks")
        _HOOK = [None]
        mod.set_axon_ntff_profile_hook = lambda h: _HOOK.__setitem__(0, h)
        mod.get_axon_ntff_profile_hook = lambda: _HOOK[0]
        sys.modules["antenv.axon_hooks"] = mod
        antenv.axon_hooks = mod
        from trn_agent_boot.trn_boot import _ntff_profile_via_ctypes
        hook = _ntff_profile_via_ctypes('/opt/axon/libaxon_pjrt.so')
        mod.set_axon_ntff_profile_hook(hook)
    except Exception:
        pass


def _emit_layer(nc, tc, pools, l):
    (wpool, psum, usb, tmpp, rbp,
     hT, hf8, uT, vtm, expbufs, ones_dr, maskT) = pools

    # ---- per-layer weight loads ---------------------------------------
    gu_sb = wpool.tile([P, DC, D], f8, tag="gu")
    wv_sb = wpool.tile([P, DC, D], f8, tag="wv")
    w1_sb = wpool.tile([P, DC, E], f8, tag="w1")
    w2_sb = wpool.tile([P, EC, D], f8, tag="w2")
    cons = wpool.tile([P, 10], f32, tag="cons")
    rows = ds(l * P, P)
    nc.sync.dma_start(out=gu_sb, in_=nc.t_gu[rows, :])
    nc.sync.dma_start(out=wv_sb, in_=nc.t_wv[rows, :])
    nc.sync.dma_start(out=w1_sb, in_=nc.t_w1[rows, :])
    nc.sync.dma_start(out=w2_sb, in_=nc.t_w2[rows, :])
    nc.sync.dma_start(out=cons, in_=nc.t_cons[rows, :])
    # cons: 0:2 = c*AU per oc, 2:10 = 0.5*b1' per ec

    # ---- cast h -> fp8 (phase A): gpsimd software-DGE DMA cast --------
    for tt in range(TT):
        tsl = slice(tt * 512, (tt + 1) * 512)
        nc.gpsimd.dma_start(out=hf8[:, :, tsl], in_=hT[:, :, tsl])

    # ---- u projection (merged QK) + V ---------------------------------
    for tt in range(TT):
        tsl = slice(tt * 512, (tt + 1) * 512)
        pu2 = psum.tile([P, 2, 512], f32, tag="bank2")
        for oc in range(DC):
            nc.tensor.matmul(pu2[:, oc, :],
                             gu_sb[:, :, oc * P:(oc + 1) * P], hf8[:, :, tsl],
                             start=True, stop=True, perf_mode=DR)
        for oc in range(DC):
            nc.vector.tensor_scalar(uT[:, oc, tsl], pu2[:, oc, :],
                                    SU_DR, cons[:, oc:oc + 1],
                                    op0=ALU.mult, op1=ALU.add)
    for tbq in range(TB // 4):
        pv4 = psum.tile([P, 4, 256], f32, tag="bank2")
        for j in range(4):
            tb = tbq * 4 + j
            nc.tensor.matmul(pv4[:, j, :],
                             hf8[:, :, tb * P:(tb + 1) * P], wv_sb[:, :, :],
                             start=True, stop=True, perf_mode=DR)
        nc.vector.tensor_copy(out=vtm[:, tbq * 4:tbq * 4 + 4, :], in_=pv4[:, :, :])

    # ---- attention -----------------------------------------------------
    for w in range(NW):
        q0 = w * W
        expT = expbufs[w % 2]
        kstart = (w - 1) * W
        # full key blocks (kb 0..3) in pairs: no mask, batched exp
        if w > 0:
            for pair in range(2):
                sc2 = psum.tile([P, 2, 512], f32, tag="bank2")
                for j in range(2):
                    kb = pair * 2 + j
                    kpos = kstart + kb * P
                    nc.tensor.matmul(sc2[:, j, :],
                                     uT[:, :, kpos:kpos + P],
                                     hf8[:, :, q0:q0 + 512],
                                     start=True, stop=True, perf_mode=DR)
                nc.scalar.activation(expT[:, pair * 2:pair * 2 + 2, :],
                                     sc2[:, :, :], AF.Exp, scale=SEXP)
        # diagonal key blocks (kb 4..7) in pairs: per-block exp + mask
        for pair in range(2):
            sc2 = psum.tile([P, 2, 512], f32, tag="bank2")
            for j in range(2):
                kb = 4 + pair * 2 + j
                kpos = kstart + kb * P
                qlo = (kb - 4) * P
                nc.tensor.matmul(sc2[:, j, qlo:512],
                                 uT[:, :, kpos:kpos + P],
                                 hf8[:, :, q0 + qlo:q0 + 512],
                                 start=True, stop=True, perf_mode=DR)
                nc.vector.tensor_tensor(sc2[:, j, qlo:qlo + P],
                                        sc2[:, j, qlo:qlo + P],
                                        maskT[:, :], op=ALU.add)
                nc.scalar.activation(expT[:, 4 + pair * 2 + j, qlo:512],
                                     sc2[:, j, qlo:512],
                                     AF.Exp, scale=SEXP)
        # AV + denominator (DoubleRow pairs over 256-key chunks)
        acc2 = psum.tile([P, 2, 512], f32, tag="bank2")
        ssb = psum.tile([P, 2, 512], f32, tag="bank2")
        for jj in range(4):
            tb0 = max((w - 1) * 4 + 2 * jj, 0)
            first, last = (jj == 0), (jj == 3)
            esl = expT[:, 2 * jj:2 * jj + 2, :]
            for dc in range(DC):
                nc.tensor.matmul(acc2[:, dc, :],
                                 vtm[:, tb0:tb0 + 2, dc * P:(dc + 1) * P],
                                 esl, start=first, stop=last, perf_mode=DR,
                                 skip_group_check=True)
            nc.tensor.matmul(ssb[:, 0, :], ones_dr[:, :, :], esl,
                             start=first, stop=last, perf_mode=DR,
                             skip_group_check=True)
        recip = rbp.tile([P, 512], f32, tag="recip")
        nc.vector.reciprocal_approx_fast(out=recip[:], in_=ssb[:, 0, :])
        for dc in range(DC):
            tmp = tmpp.tile([P, 512], f32, tag="tmp")
            nc.vector.tensor_tensor(tmp[:], acc2[:, dc, :],
                                    recip[:], op=ALU.mult)
            nc.vector.tensor_tensor(hT[:, dc, q0:q0 + W], hT[:, dc, q0:q0 + W],
                                    tmp[:], op=ALU.add)

    # ---- cast h_mid -> fp8 (phase B) ----------------------------------
    for tt in range(TT):
        tsl = slice(tt * 512, (tt + 1) * 512)
        nc.gpsimd.dma_start(out=hf8[:, :, tsl], in_=hT[:, :, tsl])

    # ---- MLP (tt-pairs; tanh batched per 2 banks) ---------------------
    for ttp in range(TT // 2):
        u2 = usb.tile([P, EC, 1024], f8, tag="u")
        for ec in range(EC):
            pu2 = psum.tile([P, 2, 512], f32, tag="bank2")
            for j in range(2):
                tsl = slice((ttp * 2 + j) * 512, (ttp * 2 + j + 1) * 512)
                nc.tensor.matmul(pu2[:, j, :],
                                 w1_sb[:, :, ec * P:(ec + 1) * P],
                                 hf8[:, :, tsl], start=True, stop=True,
                                 perf_mode=DR)
            nc.scalar.activation(u2[:, ec, :], pu2[:, :, :], AF.Tanh,
                                 bias=cons[:, 2 + ec:3 + ec], scale=STANH)
        for j in range(2):
            tsl = slice((ttp * 2 + j) * 512, (ttp * 2 + j + 1) * 512)
            pm2 = psum.tile([P, 2, 512], f32, tag="bank2")
            for dc in range(DC):
                for jj in range(4):
                    nc.tensor.matmul(
                        pm2[:, dc, :],
                        w2_sb[:, 2 * jj:2 * jj + 2, dc * P:(dc + 1) * P],
                        u2[:, 2 * jj:2 * jj + 2, j * 512:(j + 1) * 512],
                        start=(jj == 0), stop=(jj == 3), perf_mode=DR,
                        skip_group_check=True)
            nc.vector.scalar_tensor_tensor(
                hT[:, :, tsl], pm2[:, :, :], SW2, hT[:, :, tsl],
                op0=ALU.mult, op1=ALU.add)


def _build(n_layers=L):
    nc = bacc.Bacc("TRN2", target_bir_lowering=False, debug=False)

    h0_d = nc.dram_tensor("h0T", [D, S], f32, kind="ExternalInput")
    nc.t_gu = nc.dram_tensor("gu8", [n_layers * P, DC * D], f8, kind="ExternalInput")
    nc.t_wv = nc.dram_tensor("wv8", [n_layers * P, DC * D], f8, kind="ExternalInput")
    nc.t_w1 = nc.dram_tensor("w18", [n_layers * P, DC * E], f8, kind="ExternalInput")
    nc.t_w2 = nc.dram_tensor("w28", [n_layers * P, EC * D], f8, kind="ExternalInput")
    nc.t_cons = nc.dram_tensor("cons", [n_layers * P, 10], f32, kind="ExternalInput")
    mask_d = nc.dram_tensor("maskT", [P, P], f32, kind="ExternalInput")
    wfT_d = nc.dram_tensor("wfT", [D, D], bf16, kind="ExternalInput")
    bf_d = nc.dram_tensor("bfc", [P, DC], f32, kind="ExternalInput")
    out_d = nc.dram_tensor("outT", [D, S], f32, kind="ExternalOutput")

    with tile.TileContext(nc) as tc:
        with tc.tile_pool(name="persist", bufs=1) as persist, \
             tc.tile_pool(name="wpool", bufs=2) as wpool, \
             tc.tile_pool(name="psum", bufs=4, space="PSUM") as psum, \
             tc.tile_pool(name="usb", bufs=2) as usb, \
             tc.tile_pool(name="tmpp", bufs=3) as tmpp, \
             tc.tile_pool(name="rbp", bufs=2) as rbp, \
             tc.tile_pool(name="outp", bufs=4) as outp:

            hT = persist.tile([P, DC, S], f32)
            hf8 = persist.tile([P, DC, S], f8)
            uT = persist.tile([P, DC, S], f8)
            vtm = persist.tile([P, TB, D], f8)
            expA = persist.tile([P, 8, 512], f8)
            expB = persist.tile([P, 8, 512], f8)
            ones_dr = persist.tile([P, DC, P], f8)
            maskT = persist.tile([P, P], f32)
            wf_sb = persist.tile([P, DC, D], bf16)
            bf_sb = persist.tile([P, DC], f32)
            hbf = persist.tile([P, DC, S], bf16)

            nc.vector.memset(ones_dr, WVS)
            nc.vector.memset(expA, 0.0)
            nc.vector.memset(expB, 0.0)
            nc.sync.dma_start(out=maskT, in_=mask_d[:, :])
            for kc in range(DC):
                nc.sync.dma_start(out=hT[:, kc, :], in_=h0_d[kc * P:(kc + 1) * P, :])
                nc.sync.dma_start(out=wf_sb[:, kc, :], in_=wfT_d[kc * P:(kc + 1) * P, :])
            nc.sync.dma_start(out=bf_sb, in_=bf_d[:, :])

            pools = (wpool, psum, usb, tmpp, rbp,
                     hT, hf8, uT, vtm, (expA, expB), ones_dr, maskT)

            for l in range(n_layers):
                _emit_layer(nc, tc, pools, l)

            # final 1x1 conv + relu (bf16), feature-major out
            for tt in range(TT):
                tsl = slice(tt * 512, (tt + 1) * 512)
                nc.vector.tensor_copy(out=hbf[:, :, tsl], in_=hT[:, :, tsl])
            for oc in range(DC):
                for tt in range(TT):
                    tsl = slice(tt * 512, (tt + 1) * 512)
                    pf = psum.tile([P, 2, 512], f32, tag="bank2")
                    for kc in range(DC):
                        nc.tensor.matmul(pf[:, 0, :], wf_sb[:, kc, oc * P:(oc + 1) * P],
                                         hbf[:, kc, tsl],
                                         start=(kc == 0), stop=(kc == DC - 1))
                    ot = outp.tile([P, 512], f32, tag="out")
                    nc.scalar.activation(ot[:], pf[:, 0, :], AF.Relu,
                                         bias=bf_sb[:, oc:oc + 1])
                    nc.sync.dma_start(out=out_d[oc * P:(oc + 1) * P, tsl], in_=ot[:])

    nc.compile()
    return nc


def _dr_weight(Wmat, scale):
    """[out_dim, in_dim] f64 -> fp8 [128, (in//128)*out]: [p, c, m] =
    W[m, c*128+p] * scale (DoubleRow stationary layout)."""
    out_dim, in_dim = Wmat.shape
    c = in_dim // P
    w = (Wmat.T * scale).astype(np.float32)
    w = w.reshape(c, P, out_dim).transpose(1, 0, 2)
    return np.ascontiguousarray(w.reshape(P, c * out_dim)).astype(
        ml_dtypes.float8_e4m3)


def _prep_host(inputs, n_layers=L):
    bfl = ml_dtypes.bfloat16
    x = np.asarray(inputs['x'])
    emb = np.asarray(inputs['emb'], np.float32)
    bn_scale = 1.0 / np.sqrt(1.0 + BN_EPS)

    Wq = np.asarray(inputs['Wq'], np.float64)[:n_layers]
    Wk = np.asarray(inputs['Wk'], np.float64)[:n_layers]
    Wv = np.asarray(inputs['Wv'], np.float64)[:n_layers]
    W1 = np.asarray(inputs['W1'], np.float64)[:n_layers]
    W2 = np.asarray(inputs['W2'], np.float64)[:n_layers]
    bq = np.asarray(inputs['bq'], np.float64)[:n_layers]
    bv = np.asarray(inputs['bv'], np.float64)[:n_layers]
    b1 = np.asarray(inputs['b1'], np.float64)[:n_layers]
    b2 = np.asarray(inputs['b2'], np.float64)[:n_layers]
    gamma = np.asarray(inputs['gamma'], np.float64)[:n_layers]
    beta = np.asarray(inputs['beta'], np.float64)[:n_layers]

    gu8 = np.empty((n_layers * P, DC * D), ml_dtypes.float8_e4m3)
    wv8 = np.empty_like(gu8)
    w18 = np.empty((n_layers * P, DC * E), ml_dtypes.float8_e4m3)
    w28 = np.empty((n_layers * P, EC * D), ml_dtypes.float8_e4m3)
    cons = np.zeros((n_layers, P, 10), np.float32)

    Pv = np.ones(D)
    Q = np.zeros(D)
    for l in range(n_layers):
        A = gamma[l] * bn_scale
        C = beta[l]
        rows = slice(l * P, (l + 1) * P)
        Wqf = Wq[l] * Pv[None, :]
        Wkf = Wk[l] * Pv[None, :]
        bq_f = Wq[l] @ Q + bq[l]
        G = (Wkf.T @ Wqf) / 16.0              # [in_key, in_query]
        c = (Wkf.T @ bq_f) / 16.0
        gu8[rows] = _dr_weight(G.T, WG)       # u = G^T h  (Wu = G^T)
        wv8[rows] = _dr_weight(Wv[l] * Pv[None, :] / Pv[:, None], WVS)
        cv0 = Wv[l] @ Q + bv[l]
        Qmid = Q + cv0
        w18[rows] = _dr_weight(W1[l] * Pv[None, :], WS)
        w28[rows] = _dr_weight(0.5 * W2[l] / Pv[:, None], WS)
        b1_f = (W1[l] @ Qmid + b1[l])
        cons[l, :, 0:2] = (c * AU).reshape(DC, P).T
        cons[l, :, 2:10] = (0.5 * b1_f).reshape(EC, P).T
        Q = A * (Qmid + 0.5 * W2[l].sum(axis=1) + b2[l]) + C
        Pv = A * Pv

    r = np.arange(P)
    maskT = np.where(r[None, :] >= r[:, None], 0.0, NEG).astype(np.float32)

    Wf = np.asarray(inputs['Wf'], np.float64)
    bfv = np.asarray(inputs['bf'], np.float64)
    wfT = np.ascontiguousarray((Wf * Pv[None, :]).T.astype(np.float32)).astype(bfl)
    bf_f = (Wf @ Q + bfv).astype(np.float32)
    bfc = bf_f.reshape(DC, P).T.copy()

    shared = dict(gu8=gu8, wv8=wv8, w18=w18, w28=w28,
                  cons=cons.reshape(n_layers * P, 10),
                  maskT=maskT, wfT=wfT, bfc=bfc)

    h0 = emb[x]
    in_maps = []
    for b in range(B):
        m = dict(shared)
        m['h0T'] = np.ascontiguousarray(h0[b].T)
        in_maps.append(m)
    return in_maps


def kernel(**inputs):
    global LAST_EXEC_NS, LAST_TRACE
    n_layers = int(os.environ.get('KERNEL_NLAYERS', L))
    trace = os.environ.get('KERNEL_TRACE', '0') == '1'
    if trace:
        _install_ntff_hook()

    key = n_layers
    if key not in _cache:
        _cache[key] = _build(n_layers=n_layers)
    nc = _cache[key]

    in_maps = _prep_host(inputs, n_layers=n_layers)
    res = run_bass_kernel_spmd(nc, in_maps, core_ids=list(range(B)), trace=trace)
    LAST_EXEC_NS = res.exec_time_ns
    LAST_TRACE = res.instructions_and_trace[1] if res.instructions_and_trace else None
    out = np.stack([res.results[b]['outT'] for b in range(B)], axis=0)
    return out
